# revision 1
# baseline (speedup 1.0000x reference)
"""BP LDPC decoder as a sparse/segment Trainium2 kernel.

The reference multiplies dense [E,E] (E=3456) "exclusive sum" operators every
iteration.  Those operators are just the check-node / variable-node exclusive
sums of a (DV=3)-regular LDPC graph, so here the whole iteration is done with
per-edge elementwise ops + two indirect DMAs (scatter per-edge values into a
check-padded table, reduce, gather the per-check totals back per edge).  The
E x E / E x N matrices never touch the device.

Sharding: pure data parallel over the batch (16 -> 2 samples on each of the
8 cores); the graph structure (offset tables) is replicated.
"""

import sys
import numpy as np

for _p in ("/opt/trn_rl_repo", "/root/.axon_site/_ro/trn_rl_repo"):
    if _p not in sys.path:
        sys.path.insert(0, _p)

N = 1152          # variables
E = 3456          # edges (DV=3 per variable)
B = 16            # batch
NCORES = 8
BP = B // NCORES  # batch per core
NQ = N // 128     # variables per partition
M2 = 640          # padded check count (128 * 5)
S = M2 // 128     # checks per partition
DMAX = 16         # max check degree
RC = 2            # row channels: (lt, b)

_A32 = float(np.float32(1.0 + 1e-8))
_B32 = float(np.float32(1.0 - 1e-8))


def _derive_structure(H_sumC_to_V, H_xe_v_sumc_to_y):
    """Recover the LDPC graph from the dense operators.

    Device edge order is col-major (variable-major): e = 3*v + j, with
    variable v on partition v // NQ.  Returns int32 offset tables indexed by
    device edge order:
      offs_scatter[e] = row in the check-padded table T1 (= c*DMAX + k)
      offs_tot[e]     = row in the per-check totals table T2 (= c)
    """
    H_sumC_to_V = np.asarray(H_sumC_to_V)
    H_xe_v_sumc_to_y = np.asarray(H_xe_v_sumc_to_y)
    cols_rm = np.argmax(H_xe_v_sumc_to_y, axis=0)        # variable of row-major edge
    p_r2l = np.argsort(cols_rm, kind="stable")           # col-major pos -> row-major idx
    p_l2r = np.argsort(p_r2l)
    Br = H_sumC_to_V[:, p_l2r]                           # same-check adjacency (row-major)
    same = Br[np.arange(E - 1), np.arange(1, E)] > 0
    check_id = np.concatenate([[0], np.cumsum(~same)]).astype(np.int64)
    deg = np.bincount(check_id)
    assert deg.max() <= DMAX, f"check degree {deg.max()} > {DMAX}"
    assert check_id[-1] < M2
    start = np.concatenate([[0], np.cumsum(deg)[:-1]])
    k_slot = np.arange(E) - start[check_id]
    r = p_r2l                                            # row-major index of device edge e
    # device edge order: e = 3v+j = (p*9+q)*3+j -> [128, 27] (partition, slot)
    offs_scatter = (check_id[r] * DMAX + k_slot[r]).astype(np.int32).reshape(128, 27)
    offs_tot = check_id[r].astype(np.int32).reshape(128, 27)
    return offs_scatter, offs_tot


def _build_program(n_iters: int):
    import concourse.bacc as bacc
    import concourse.hw_specs as hw_specs
    # Force every activation onto the one table set containing all our
    # functions (ln/exp/sign/abs); otherwise the chooser ping-pongs between
    # sets and reloads tables 4x per iteration (~41us of ACT_TABLE_LOAD).
    if not getattr(bacc, "_act_tables_pinned", False):
        _orig_get = hw_specs.get_activation_tables

        def _pinned(arch):
            tabs = _orig_get(arch)
            keep = "natural_log_exp_and_others"
            if keep in tabs:
                tabs = {k: (v if k == keep else set()) for k, v in tabs.items()}
            return tabs

        bacc.get_activation_tables = _pinned
        bacc._act_tables_pinned = True
    import concourse.mybir as mybir
    import concourse.tile as tile
    from concourse.bass import IndirectOffsetOnAxis

    f32 = mybir.dt.float32
    i32 = mybir.dt.int32
    AF = mybir.ActivationFunctionType
    ALU = mybir.AluOpType

    nc = bacc.Bacc("TRN2", target_bir_lowering=False, debug=False)

    llr_d = nc.declare_dram_parameter("llr", [BP, N], f32, isOutput=False)
    osc_d = nc.declare_dram_parameter("offs_scatter", [128, 27], i32, isOutput=False)
    ott_d = nc.declare_dram_parameter("offs_tot", [128, 27], i32, isOutput=False)
    dec_d = nc.declare_dram_parameter("dec", [BP, N], i32, isOutput=True)

    T1 = nc.dram_tensor("T1", [M2 * DMAX, RC * BP], f32)   # check-padded per-edge rows
    T2 = nc.dram_tensor("T2", [M2, RC * BP], f32)          # per-check totals

    with tile.TileContext(nc) as tc:
        with tc.tile_pool(name="st", bufs=1) as st:
            llr_sb = st.tile([128, NQ, BP], f32)
            xe0 = st.tile([128, NQ, 3, BP], f32)
            x = st.tile([128, NQ, 3, BP], f32)
            offs_sc = st.tile([128, 27], i32)
            offs_tt = st.tile([128, 27], i32)
            pair = st.tile([128, NQ, 3, RC, BP], f32)      # (lt, b) per edge
            P = st.tile([128, S, DMAX, RC, BP], f32)       # check-padded gather-in
            tot = st.tile([128, S, RC, BP], f32)           # (Lt, K) per check
            G = st.tile([128, NQ, 3, RC, BP], f32)         # per-edge totals
            d = st.tile([128, NQ, 3, RC, BP], f32)         # (sr, kx) per edge
            ax = st.tile([128, NQ, 3, BP], f32)
            u = st.tile([128, NQ, 3, BP], f32)
            lnum = st.tile([128, NQ, 3, BP], f32)
            lden = st.tile([128, NQ, 3, BP], f32)
            xp = st.tile([128, NQ, 3, BP], f32)
            m = st.tile([128, NQ, 3, BP], f32)
            sgnx = st.tile([128, NQ, 3, BP], f32)
            w = st.tile([128, NQ, 3, BP], f32)
            pd = st.tile([128, NQ, 3, BP], f32)
            ki = st.tile([128, NQ, 3, BP], i32)
            kb = st.tile([128, NQ, 3, BP], i32)
            num = st.tile([128, NQ, 3, BP], f32)
            den = st.tile([128, NQ, 3, BP], f32)
            y = st.tile([128, NQ, 3, BP], f32)
            V = st.tile([128, NQ, BP], f32)
            bp_t = st.tile([128, NQ, BP], f32)
            sg = st.tile([128, NQ, BP], f32)
            dec_f = st.tile([128, NQ, BP], f32)
            dec_i = st.tile([128, NQ, BP], i32)
            zeros = st.tile([128, S * DMAX * RC * BP], f32)

            # ---- init ----
            nc.sync.dma_start(
                out=llr_sb[:], in_=llr_d.ap().rearrange("b (p q) -> p q b", p=128)
            )
            nc.sync.dma_start(out=offs_sc[:], in_=osc_d.ap())
            nc.sync.dma_start(out=offs_tt[:], in_=ott_d.ap())
            nc.vector.memset(zeros[:], 0.0)
            nc.sync.dma_start(
                out=T1.ap().rearrange("(p r) c -> p (r c)", p=128), in_=zeros[:]
            )
            bc = llr_sb[:].unsqueeze(2).to_broadcast([128, NQ, 3, BP])
            nc.vector.tensor_copy(xe0[:], bc)
            nc.vector.tensor_copy(x[:], xe0[:])

            pair_lt = pair[:, :, :, 0, :]
            pair_b = pair[:, :, :, 1, :]

            for t in range(n_iters):
                # lt = ln(1e-8 + tanh(|x|/2)) computed exp/ln-only:
                #   u = exp(-|x|); lt = ln(A - B*u) - ln(1 + u)
                nc.scalar.activation(ax[:], x[:], AF.Abs)
                nc.scalar.activation(u[:], ax[:], AF.Exp, scale=-1.0)
                nc.scalar.activation(lnum[:], u[:], AF.Ln, bias=_A32, scale=-_B32)
                nc.scalar.activation(lden[:], u[:], AF.Ln, bias=1.0)
                nc.vector.tensor_tensor(pair_lt, lnum[:], lden[:], ALU.subtract)
                # b = 1 if x < 0 else 0   (sign bookkeeping for the check product)
                nc.vector.tensor_scalar(pair_b, x[:], 0.0, None, ALU.is_lt)

                # scatter per-edge (lt, b) rows into the check-padded table
                # (verified primitive: one run per partition per DMA)
                pairw = pair[:].rearrange("p a b c d -> p (a b) (c d)")
                for sl in range(27):
                    nc.gpsimd.indirect_dma_start(
                        out=T1.ap(),
                        out_offset=IndirectOffsetOnAxis(ap=offs_sc[:, sl:sl + 1], axis=0),
                        in_=pairw[:, sl, :],
                        in_offset=None,
                    )
                # dense load back as [check, slot] and reduce over slots
                nc.sync.dma_start(
                    out=P[:].rearrange("p a b c d -> p (a b c d)"),
                    in_=T1.ap().rearrange("(p r) c -> p (r c)", p=128),
                )
                nc.vector.tensor_reduce(
                    tot[:], P[:].transpose([0, 1, 3, 4, 2]),
                    axis=mybir.AxisListType.X, op=ALU.add,
                )
                # totals out to T2, gather back per edge
                nc.sync.dma_start(
                    out=T2.ap().rearrange("(p s) c -> p (s c)", p=128), in_=tot[:]
                )
                Gw = G[:].rearrange("p a b c d -> p (a b) (c d)")
                for sl in range(27):
                    nc.gpsimd.indirect_dma_start(
                        out=Gw[:, sl, :],
                        out_offset=None,
                        in_=T2.ap(),
                        in_offset=IndirectOffsetOnAxis(ap=offs_tt[:, sl:sl + 1], axis=0),
                    )

                # exclusive check sums: sr = Lt - lt, kx = K - b (packed)
                nc.vector.tensor_tensor(d[:], G[:], pair[:], ALU.subtract)
                nc.scalar.activation(xp[:], d[:, :, :, 0, :], AF.Exp)
                # sign of exclusive product: (-1)^kx
                nc.vector.tensor_copy(ki[:], d[:, :, :, 1, :])
                nc.vector.tensor_scalar(kb[:], ki[:], 1, None, ALU.bitwise_and)
                nc.vector.tensor_scalar(sgnx[:], kb[:], -2.0, 1.0, ALU.mult, ALU.add)
                nc.vector.tensor_scalar(w[:], xp[:], -2e-7, None, ALU.add)
                nc.vector.tensor_tensor(pd[:], w[:], sgnx[:], ALU.mult)
                # y = ln((1 + pd) / (1 - pd + 1e-10))
                nc.scalar.activation(num[:], pd[:], AF.Ln, bias=1.0)
                nc.scalar.activation(den[:], pd[:], AF.Ln, bias=float(np.float32(1.0 + 1e-10)), scale=-1.0)
                nc.vector.tensor_tensor(y[:], num[:], den[:], ALU.subtract)

                # variable-node side is local: V = sum_j y
                nc.vector.tensor_reduce(
                    V[:], y[:].transpose([0, 1, 3, 2]),
                    axis=mybir.AxisListType.X, op=ALU.add,
                )
                if t < n_iters - 1:
                    nc.vector.tensor_tensor(w[:], xe0[:], y[:], ALU.subtract)
                    nc.vector.tensor_tensor(
                        x[:], w[:],
                        V[:].unsqueeze(2).to_broadcast([128, NQ, 3, BP]), ALU.add,
                    )
                else:
                    nc.vector.tensor_tensor(bp_t[:], llr_sb[:], V[:], ALU.add)
                    nc.scalar.activation(sg[:], bp_t[:], AF.Sign)
                    nc.vector.tensor_scalar(dec_f[:], sg[:], -0.5, 0.5, ALU.mult, ALU.add)
                    nc.vector.tensor_copy(dec_i[:], dec_f[:])
                    nc.sync.dma_start(
                        out=dec_d.ap().rearrange("b (p q) -> p q b", p=128),
                        in_=dec_i[:],
                    )
    nc.compile()
    return nc


_PROGRAM_CACHE = {}


def _get_program(n_iters: int):
    if n_iters not in _PROGRAM_CACHE:
        _PROGRAM_CACHE[n_iters] = _build_program(n_iters)
    return _PROGRAM_CACHE[n_iters]


def _make_in_maps(llr_in, H_sumC_to_V, H_xe_v_sumc_to_y):
    llr = np.ascontiguousarray(np.asarray(llr_in, dtype=np.float32))
    assert llr.shape == (B, N)
    offs_sc, offs_tt = _derive_structure(H_sumC_to_V, H_xe_v_sumc_to_y)
    osc = np.ascontiguousarray(offs_sc)
    ott = np.ascontiguousarray(offs_tt)
    return [
        {
            "llr": np.ascontiguousarray(llr[c * BP:(c + 1) * BP]),
            "offs_scatter": osc,
            "offs_tot": ott,
        }
        for c in range(NCORES)
    ]


def kernel(llr_in, H_x_to_xe0, H_sumC_to_V, H_sumV_to_C, H_xe_v_sumc_to_y,
           bp_iter_num, **_unused):
    from concourse.bass_utils import run_bass_kernel_spmd

    n_iters = int(np.asarray(bp_iter_num))
    nc = _get_program(n_iters)
    in_maps = _make_in_maps(llr_in, H_sumC_to_V, H_xe_v_sumc_to_y)
    res = run_bass_kernel_spmd(nc, in_maps, list(range(NCORES)))
    out = np.concatenate([res.results[c]["dec"] for c in range(NCORES)], axis=0)
    return out.astype(np.int32)



# revision 6
# speedup vs baseline: 3.9374x; 3.9374x over previous
"""BP LDPC decoder as an on-chip PE-routed Trainium2 kernel.

The reference multiplies dense [E,E] (E=3456) "exclusive sum" operators every
iteration.  Those operators are the check-node / variable-node exclusive sums
of a (DV=3)-regular LDPC graph.  The variable-node side is local to a
partition-major edge layout; the check-node side (a fixed graph permutation +
segmented reduce) is done entirely on the tensor engine with 0/1 routing
matrices held in SBUF:

  scatter: tot[pc, g, :] = sum_s A_s^T @ (pair[:, s] masked into its g slot)
  gather:  U[pe, s, g, :] = B_s^T @ tot ; G = sum_g U * mask

so nothing round-trips through DRAM inside the iteration loop (the baseline
did 54 indirect DMAs per iteration through HBM; that was ~90% of its time).

Sharding: pure data parallel over the batch (16 -> 2 samples per core); the
graph structure (routing matrices) is replicated.
"""

import sys
import numpy as np

for _p in ("/opt/trn_rl_repo", "/root/.axon_site/_ro/trn_rl_repo"):
    if _p not in sys.path:
        sys.path.insert(0, _p)

N = 1152          # variables
E = 3456          # edges (DV=3 per variable)
B = 16            # batch
NCORES = 8
BP = B // NCORES  # batch per core
NQ = N // 128     # variables per partition (9)
NS = E // 128     # edge slots per partition (27)
NG = 5            # check groups of 128 (576 checks -> 5*128 slots)
RC = 2            # row channels: (lt, b)
FW = RC * BP      # matmul free width per (s, g)

_A32 = float(np.float32(1.0 + 1e-8))
_B32 = float(np.float32(1.0 - 1e-8))


def _derive_structure(H_sumC_to_V, H_xe_v_sumc_to_y):
    """Recover the LDPC graph and build the PE routing operators.

    Device edge order is col-major (variable-major): e = 3*v + j, variable v
    on partition v // NQ, slot s = e % NS.  Check c sits at PSUM row c % 128
    in group c // 128.
    Returns:
      wA [128, NS, 128] f32: wA[pe, s, pc] = 1 iff edge (pe,s) -> check row pc
      wB [128, NS, 128] f32: wB[pc, s, pe] = wA[pe, s, pc]   (gather routing)
      mk [128, NS, NG] f32:  mk[pe, s, g]  = 1 iff edge (pe,s) -> group g
    """
    H_sumC_to_V = np.asarray(H_sumC_to_V)
    H_xe_v_sumc_to_y = np.asarray(H_xe_v_sumc_to_y)
    cols_rm = np.argmax(H_xe_v_sumc_to_y, axis=0)        # variable of row-major edge
    p_r2l = np.argsort(cols_rm, kind="stable")           # col-major pos -> row-major idx
    p_l2r = np.argsort(p_r2l)
    Br = H_sumC_to_V[:, p_l2r]                           # same-check adjacency (row-major)
    same = Br[np.arange(E - 1), np.arange(1, E)] > 0
    check_id = np.concatenate([[0], np.cumsum(~same)]).astype(np.int64)
    ck = check_id[p_r2l]                                 # check of device edge e
    assert ck.max() < NG * 128
    pc = (ck % 128).astype(np.int64).reshape(128, NS)    # [pe, s]
    g = (ck // 128).astype(np.int64).reshape(128, NS)
    pe_idx = np.arange(128)[:, None].repeat(NS, 1)
    s_idx = np.arange(NS)[None, :].repeat(128, 0)
    wA = np.zeros((128, NS, 128), np.float32)
    wA[pe_idx, s_idx, pc] = 1.0
    wB = np.zeros((128, NS, 128), np.float32)
    wB[pc, s_idx, pe_idx] = 1.0
    mk = np.zeros((128, NS, NG), np.float32)
    mk[pe_idx, s_idx, g] = 1.0
    return wA, wB, mk


def _build_program(n_iters: int):
    import concourse.bacc as bacc
    import concourse.hw_specs as hw_specs
    # Force every activation onto the one table set containing all our
    # functions (ln/exp/sign/abs); otherwise the chooser ping-pongs between
    # sets and reloads tables (~2.7us per reload).
    if not getattr(bacc, "_act_tables_pinned", False):
        _orig_get = hw_specs.get_activation_tables

        def _pinned(arch):
            tabs = _orig_get(arch)
            keep = "natural_log_exp_and_others"
            if keep in tabs:
                tabs = {k: (v if k == keep else set()) for k, v in tabs.items()}
            return tabs

        bacc.get_activation_tables = _pinned
        bacc._act_tables_pinned = True
    import concourse.mybir as mybir
    import concourse.tile as tile

    f32 = mybir.dt.float32
    i32 = mybir.dt.int32
    AF = mybir.ActivationFunctionType
    ALU = mybir.AluOpType

    nc = bacc.Bacc("TRN2", target_bir_lowering=False, debug=False)

    llr_d = nc.declare_dram_parameter("llr", [BP, N], f32, isOutput=False)
    wA_d = nc.declare_dram_parameter("wA", [128, NS * 128], f32, isOutput=False)
    wB_d = nc.declare_dram_parameter("wB", [128, NS * 128], f32, isOutput=False)
    mk_d = nc.declare_dram_parameter("mk", [128, NS * NG], f32, isOutput=False)
    dec_d = nc.declare_dram_parameter("dec", [BP, N], i32, isOutput=True)

    with tile.TileContext(nc) as tc:
        with tc.tile_pool(name="st", bufs=1) as st, \
             tc.tile_pool(name="ps", bufs=1, space="PSUM") as ps:
            wA = st.tile([128, NS, 128], f32)
            wB = st.tile([128, NS, 128], f32)
            mk = st.tile([128, NS, NG, 1, 1], f32)
            llr_sb = st.tile([128, NQ, BP], f32)
            x = st.tile([128, NQ, 3, BP], f32)         # per-edge v->c messages
            pair = st.tile([128, NS, RC, BP], f32)     # (lt, b) per edge
            R = st.tile([128, NS, NG, RC, BP], f32)    # mask-expanded rhs
            tot_sb = st.tile([128, NG, RC, BP], f32)   # per-check (Lt, K)
            Um = st.tile([128, NS, NG, RC, BP], f32)
            G = st.tile([128, NS, RC, BP], f32)        # gathered per-edge totals
            d = st.tile([128, NS, RC, BP], f32)        # exclusive (sr, kx)
            ax = st.tile([128, NQ, 3, BP], f32)
            u = st.tile([128, NQ, 3, BP], f32)
            lnum = st.tile([128, NQ, 3, BP], f32)
            lden = st.tile([128, NQ, 3, BP], f32)
            xp = st.tile([128, NQ, 3, BP], f32)
            ki = st.tile([128, NQ, 3, BP], i32)
            kb = st.tile([128, NQ, 3, BP], i32)
            sgnx = st.tile([128, NQ, 3, BP], f32)
            w = st.tile([128, NQ, 3, BP], f32)
            pd = st.tile([128, NQ, 3, BP], f32)
            L1 = st.tile([128, NQ, 3, BP], f32)
            L2 = st.tile([128, NQ, 3, BP], f32)
            yv = st.tile([128, NQ, 4, BP], f32)        # slots 0..2 = y_j, 3 = llr
            Vf = st.tile([128, NQ, BP], f32)           # llr + sum_j y_j
            sg = st.tile([128, NQ, BP], f32)
            dec_f = st.tile([128, NQ, BP], f32)
            dec_i = st.tile([128, NQ, BP], i32)

            tot_ps = ps.tile([128, NG, RC, BP], f32)   # 80B -> 1 bank
            U_ps = ps.tile([128, NS, 32], f32)         # 32-f32 stride: bank-aligned

            # ---- init ----
            nc.sync.dma_start(out=wA[:], in_=wA_d.ap())
            nc.sync.dma_start(out=wB[:], in_=wB_d.ap())
            nc.sync.dma_start(out=mk[:], in_=mk_d.ap())
            nc.sync.dma_start(
                out=llr_sb[:], in_=llr_d.ap().rearrange("b (p q) -> p q b", p=128)
            )
            nc.vector.tensor_copy(
                x[:], llr_sb[:].unsqueeze(2).to_broadcast([128, NQ, 3, BP])
            )
            nc.vector.tensor_copy(yv[:, :, 3, :], llr_sb[:])

            xe = x[:].rearrange("p q j b -> p (q j) b")    # [128, NS, BP] view
            pair_lt = pair[:, :, 0, :]
            pair_b = pair[:, :, 1, :]

            for t in range(n_iters):
                # lt = ln(1e-8 + tanh(|x|/2)) computed exp/ln-only:
                #   u = exp(-|x|); lt = ln(A - B*u) - ln(1 + u)
                nc.scalar.activation(ax[:], x[:], AF.Abs)
                nc.scalar.activation(u[:], ax[:], AF.Exp, scale=-1.0)
                nc.scalar.activation(lnum[:], u[:], AF.Ln, bias=_A32, scale=-_B32)
                nc.scalar.activation(lden[:], u[:], AF.Ln, bias=1.0)
                nc.vector.tensor_tensor(
                    pair_lt.rearrange("p (q j) b -> p q j b", q=NQ),
                    lnum[:], lden[:], ALU.subtract)
                # b = 1 if x < 0 else 0   (sign bookkeeping for the check product)
                nc.vector.tensor_scalar(
                    pair_b.rearrange("p (q j) b -> p q j b", q=NQ),
                    x[:], 0.0, None, ALU.is_lt)

                # mask-expand the per-edge rows into their check-group slot
                nc.vector.tensor_tensor(
                    R[:],
                    pair[:].unsqueeze(2).to_broadcast([128, NS, NG, RC, BP]),
                    mk[:].to_broadcast([128, NS, NG, RC, BP]),
                    ALU.mult)
                # scatter: tot[pc, g] = sum over edges of check (pc, g)
                for s in range(NS):
                    nc.tensor.matmul(
                        tot_ps[:],
                        wA[:, s, :],
                        R[:, s].rearrange("p g r b -> p (g r b)"),
                        start=(s == 0), stop=(s == NS - 1),
                    )
                nc.scalar.activation(tot_sb[:], tot_ps[:], AF.Copy)
                # gather: U[pe, s, g] = tot[pc(pe,s), g]
                tot_flat = tot_sb[:].rearrange("p g r b -> p (g r b)")
                for s in range(NS):
                    nc.tensor.matmul(
                        U_ps[:, s, 0:NG * FW], wB[:, s, :], tot_flat,
                        start=True, stop=True,
                    )
                Uv = U_ps[:, :, 0:NG * FW].rearrange(
                    "p s (g r b) -> p s g r b", g=NG, r=RC)
                nc.vector.tensor_tensor(
                    Um[:], Uv, mk[:].to_broadcast([128, NS, NG, RC, BP]), ALU.mult)
                nc.vector.tensor_reduce(
                    G[:], Um[:].transpose([0, 1, 3, 4, 2]),
                    axis=mybir.AxisListType.X, op=ALU.add)

                # exclusive check sums: sr = Lt - lt, kx = K - b (packed)
                nc.vector.tensor_tensor(d[:], G[:], pair[:], ALU.subtract)
                nc.scalar.activation(
                    xp[:], d[:, :, 0, :].rearrange("p (q j) b -> p q j b", q=NQ),
                    AF.Exp)
                # sign of exclusive product: (-1)^kx
                nc.vector.tensor_copy(
                    ki[:], d[:, :, 1, :].rearrange("p (q j) b -> p q j b", q=NQ))
                nc.vector.tensor_scalar(kb[:], ki[:], 1, None, ALU.bitwise_and)
                nc.vector.tensor_scalar(sgnx[:], kb[:], -2.0, 1.0, ALU.mult, ALU.add)
                # y = ln(1 + pd) - ln(1 - pd), pd = sgn * (xp - 2e-7)
                nc.vector.tensor_scalar(w[:], xp[:], -2e-7, None, ALU.add)
                nc.vector.tensor_tensor(pd[:], w[:], sgnx[:], ALU.mult)
                nc.scalar.activation(L1[:], pd[:], AF.Ln, bias=1.0)
                nc.scalar.activation(L2[:], pd[:], AF.Ln, bias=1.0, scale=-1.0)
                nc.vector.tensor_tensor(yv[:, :, 0:3, :], L1[:], L2[:], ALU.subtract)

                # variable side is local: Vf = llr + sum_j y_j
                nc.vector.tensor_reduce(
                    Vf[:], yv[:].transpose([0, 1, 3, 2]),
                    axis=mybir.AxisListType.X, op=ALU.add)
                if t < n_iters - 1:
                    nc.vector.tensor_tensor(
                        x[:],
                        Vf[:].unsqueeze(2).to_broadcast([128, NQ, 3, BP]),
                        yv[:, :, 0:3, :], ALU.subtract)
                else:
                    nc.scalar.activation(sg[:], Vf[:], AF.Sign)
                    nc.vector.tensor_scalar(dec_f[:], sg[:], -0.5, 0.5,
                                            ALU.mult, ALU.add)
                    nc.vector.tensor_copy(dec_i[:], dec_f[:])
                    nc.sync.dma_start(
                        out=dec_d.ap().rearrange("b (p q) -> p q b", p=128),
                        in_=dec_i[:],
                    )
    nc.compile()
    return nc


_PROGRAM_CACHE = {}


def _get_program(n_iters: int):
    if n_iters not in _PROGRAM_CACHE:
        _PROGRAM_CACHE[n_iters] = _build_program(n_iters)
    return _PROGRAM_CACHE[n_iters]


def _make_in_maps(llr_in, H_sumC_to_V, H_xe_v_sumc_to_y):
    llr = np.ascontiguousarray(np.asarray(llr_in, dtype=np.float32))
    assert llr.shape == (B, N)
    wA, wB, mk = _derive_structure(H_sumC_to_V, H_xe_v_sumc_to_y)
    wA = np.ascontiguousarray(wA.reshape(128, NS * 128))
    wB = np.ascontiguousarray(wB.reshape(128, NS * 128))
    mk = np.ascontiguousarray(mk.reshape(128, NS * NG))
    return [
        {
            "llr": np.ascontiguousarray(llr[c * BP:(c + 1) * BP]),
            "wA": wA,
            "wB": wB,
            "mk": mk,
        }
        for c in range(NCORES)
    ]


def kernel(llr_in, H_x_to_xe0, H_sumC_to_V, H_sumV_to_C, H_xe_v_sumc_to_y,
           bp_iter_num, **_unused):
    from concourse.bass_utils import run_bass_kernel_spmd

    n_iters = int(np.asarray(bp_iter_num))
    nc = _get_program(n_iters)
    in_maps = _make_in_maps(llr_in, H_sumC_to_V, H_xe_v_sumc_to_y)
    res = run_bass_kernel_spmd(nc, in_maps, list(range(NCORES)))
    out = np.concatenate([res.results[c]["dec"] for c in range(NCORES)], axis=0)
    return out.astype(np.int32)


# revision 10
# speedup vs baseline: 8.8838x; 2.2563x over previous
"""BP LDPC decoder as an on-chip PE-routed Trainium2 kernel.

The reference multiplies dense [E,E] (E=3456) "exclusive sum" operators every
iteration.  Those operators are the check-node / variable-node exclusive sums
of a (DV=3)-regular LDPC graph.  The variable-node side is local to a
partition-major edge layout; the check-node side (a fixed graph permutation +
segmented reduce) is done entirely on the tensor engine with 0/1 routing
matrices held in SBUF:

  scatter: tot[pc, g, :] = sum_s A_s^T @ (pair[:, s] masked into its g slot)
  gather:  U[pe, s, g, :] = B_s^T @ tot ; G = sum_g U * mask

so nothing round-trips through DRAM inside the iteration loop (the baseline
did 54 indirect DMAs per iteration through HBM; that was ~90% of its time).

The routed channels are fp16 (fp32 matmuls run 2 PE passes; fp16 runs one and
gets fast weight load).  The log-magnitude channel is carried as an fp16
hi/lo pair (lo = lt - fp16(lt)), so the reconstructed check sums match fp32
to ~2^-22 relative -- the decoded bits stay bit-exact vs the fp32 reference.
The sign-count channel is exact (small integers).

Sharding: pure data parallel over the batch (16 -> 2 samples per core); the
graph structure (routing matrices) is replicated.
"""

import sys
import numpy as np

for _p in ("/opt/trn_rl_repo", "/root/.axon_site/_ro/trn_rl_repo"):
    if _p not in sys.path:
        sys.path.insert(0, _p)

N = 1152          # variables
E = 3456          # edges (DV=3 per variable)
B = 16            # batch
NCORES = 8
BP = B // NCORES  # batch per core
NQ = N // 128     # variables per partition (9)
NS = E // 128     # edge slots per partition (27)
NG = 5            # check groups of 128 (576 checks -> 5*128 slots)
RC = 3            # routed channels: (lt_hi, lt_lo, b)
FW = RC * BP      # matmul free width per (s, g)

_A32 = float(np.float32(1.0 + 1e-8))
_B32 = float(np.float32(1.0 - 1e-8))


def _derive_structure(H_sumC_to_V, H_xe_v_sumc_to_y):
    """Recover the LDPC graph and build the PE routing operators.

    Device edge order is col-major (variable-major): e = 3*v + j, variable v
    on partition v // NQ, slot s = e % NS.  Check c sits at PSUM row c % 128
    in group c // 128.
    Returns:
      wA [128, NS, 128] f16: wA[pe, s, pc] = 1 iff edge (pe,s) -> check row pc
      wB [128, NS, 128] f16: wB[pc, s, pe] = wA[pe, s, pc]   (gather routing)
      mk [128, NS, NG] f32:  mk[pe, s, g]  = 1 iff edge (pe,s) -> group g
    """
    H_sumC_to_V = np.asarray(H_sumC_to_V)
    H_xe_v_sumc_to_y = np.asarray(H_xe_v_sumc_to_y)
    cols_rm = np.argmax(H_xe_v_sumc_to_y, axis=0)        # variable of row-major edge
    p_r2l = np.argsort(cols_rm, kind="stable")           # col-major pos -> row-major idx
    p_l2r = np.argsort(p_r2l)
    Br = H_sumC_to_V[:, p_l2r]                           # same-check adjacency (row-major)
    same = Br[np.arange(E - 1), np.arange(1, E)] > 0
    check_id = np.concatenate([[0], np.cumsum(~same)]).astype(np.int64)
    ck = check_id[p_r2l]                                 # check of device edge e
    assert ck.max() < NG * 128
    pc = (ck % 128).astype(np.int64).reshape(128, NS)    # [pe, s]
    g = (ck // 128).astype(np.int64).reshape(128, NS)
    pe_idx = np.arange(128)[:, None].repeat(NS, 1)
    s_idx = np.arange(NS)[None, :].repeat(128, 0)
    wA = np.zeros((128, NS, 128), np.float16)
    wA[pe_idx, s_idx, pc] = 1.0
    wB = np.zeros((128, NS, 128), np.float16)
    wB[pc, s_idx, pe_idx] = 1.0
    mk = np.zeros((128, NS, NG), np.float32)
    mk[pe_idx, s_idx, g] = 1.0
    return wA, wB, mk


def _build_program(n_iters: int):
    import concourse.bacc as bacc
    import concourse.hw_specs as hw_specs
    # Force every activation onto the one table set containing all our
    # functions (ln/exp/sign/abs); otherwise the chooser ping-pongs between
    # sets and reloads tables (~2.7us per reload).
    if not getattr(bacc, "_act_tables_pinned", False):
        _orig_get = hw_specs.get_activation_tables

        def _pinned(arch):
            tabs = _orig_get(arch)
            keep = "natural_log_exp_and_others"
            if keep in tabs:
                tabs = {k: (v if k == keep else set()) for k, v in tabs.items()}
            return tabs

        bacc.get_activation_tables = _pinned
        bacc._act_tables_pinned = True
    import concourse.mybir as mybir
    import concourse.tile as tile

    f32 = mybir.dt.float32
    f16 = mybir.dt.float16
    i32 = mybir.dt.int32
    AF = mybir.ActivationFunctionType
    ALU = mybir.AluOpType

    nc = bacc.Bacc("TRN2", target_bir_lowering=False, debug=False)

    llr_d = nc.declare_dram_parameter("llr", [BP, N], f32, isOutput=False)
    wA_d = nc.declare_dram_parameter("wA", [128, NS * 128], f16, isOutput=False)
    wB_d = nc.declare_dram_parameter("wB", [128, NS * 128], f16, isOutput=False)
    mk_d = nc.declare_dram_parameter("mk", [128, NS * NG], f32, isOutput=False)
    dec_d = nc.declare_dram_parameter("dec", [BP, N], i32, isOutput=True)

    with tile.TileContext(nc) as tc:
        with tc.tile_pool(name="st", bufs=1) as st, \
             tc.tile_pool(name="ps", bufs=1, space="PSUM") as ps:
            wA = st.tile([128, NS, 128], f16)
            wB = st.tile([128, NS, 128], f16)
            mk = st.tile([128, NS, NG, 1, 1], f32)
            mk16 = st.tile([128, NS, NG, 1, 1], f16)
            llr_sb = st.tile([128, NQ, BP], f32)
            x = st.tile([128, NQ, 3, BP], f32)         # per-edge v->c messages
            lt_f = st.tile([128, NS, BP], f32)         # fp32 log-magnitude
            pair = st.tile([128, NS, RC, BP], f16)     # (lt_hi, lt_lo, b) fp16
            R = st.tile([128, NS, NG, RC, BP], f16)    # mask-expanded rhs
            tbf = st.tile([128, NG, RC, BP], f16)      # gather rhs (hi, lo, K)
            tot_f = st.tile([128, NG, RC, BP], f32)
            Lt_f = st.tile([128, NG, BP], f32)
            Um = st.tile([128, NS, NG, RC, BP], f32)
            G3 = st.tile([128, NS, RC, BP], f32)       # gathered (hi, lo, K) sums
            S = st.tile([128, NQ, 3, BP], f32)
            dlt = st.tile([128, NQ, 3, BP], f32)
            db = st.tile([128, NQ, 3, BP], f32)
            ax = st.tile([128, NQ, 3, BP], f32)
            u = st.tile([128, NQ, 3, BP], f32)
            lnum = st.tile([128, NQ, 3, BP], f32)
            lden = st.tile([128, NQ, 3, BP], f32)
            xp = st.tile([128, NQ, 3, BP], f32)
            ki = st.tile([128, NQ, 3, BP], i32)
            kb = st.tile([128, NQ, 3, BP], i32)
            sgnx = st.tile([128, NQ, 3, BP], f32)
            w = st.tile([128, NQ, 3, BP], f32)
            pd = st.tile([128, NQ, 3, BP], f32)
            L1 = st.tile([128, NQ, 3, BP], f32)
            L2 = st.tile([128, NQ, 3, BP], f32)
            yv = st.tile([128, NQ, 4, BP], f32)        # slots 0..2 = y_j, 3 = llr
            Vf = st.tile([128, NQ, BP], f32)           # llr + sum_j y_j
            sg = st.tile([128, NQ, BP], f32)
            dec_f = st.tile([128, NQ, BP], f32)
            dec_i = st.tile([128, NQ, BP], i32)

            tot_ps = ps.tile([128, NG, RC, BP], f32)   # 120B -> 1 bank
            U_ps = ps.tile([128, NS, 32], f32)         # 32-f32 stride: bank-aligned

            # ---- init ----
            nc.sync.dma_start(out=wA[:], in_=wA_d.ap())
            nc.sync.dma_start(out=wB[:], in_=wB_d.ap())
            nc.sync.dma_start(out=mk[:], in_=mk_d.ap())
            nc.sync.dma_start(
                out=llr_sb[:], in_=llr_d.ap().rearrange("b (p q) -> p q b", p=128)
            )
            nc.vector.tensor_copy(mk16[:], mk[:])
            nc.vector.tensor_copy(
                x[:], llr_sb[:].unsqueeze(2).to_broadcast([128, NQ, 3, BP])
            )
            nc.vector.tensor_copy(yv[:, :, 3, :], llr_sb[:])

            pair_hi = pair[:, :, 0, :]
            pair_lo = pair[:, :, 1, :]
            pair_b = pair[:, :, 2, :]
            ltq = lt_f[:].rearrange("p (q j) b -> p q j b", q=NQ)

            for t in range(n_iters):
                # lt = ln(1e-8 + tanh(|x|/2)) computed exp/ln-only:
                #   u = exp(-|x|); lt = ln(A - B*u) - ln(1 + u)
                nc.scalar.activation(ax[:], x[:], AF.Abs)
                nc.scalar.activation(u[:], ax[:], AF.Exp, scale=-1.0)
                nc.scalar.activation(lnum[:], u[:], AF.Ln, bias=_A32, scale=-_B32)
                nc.scalar.activation(lden[:], u[:], AF.Ln, bias=1.0)
                nc.vector.tensor_tensor(ltq, lnum[:], lden[:], ALU.subtract)
                # fp16 hi/lo split of lt; b = 1 if x < 0 else 0
                nc.vector.tensor_copy(pair_hi, lt_f[:])
                nc.vector.tensor_tensor(pair_lo, lt_f[:], pair_hi, ALU.subtract)
                nc.vector.tensor_scalar(
                    pair_b.rearrange("p (q j) b -> p q j b", q=NQ),
                    x[:], 0.0, None, ALU.is_lt)

                # mask-expand the per-edge rows into their check-group slot
                nc.vector.tensor_tensor(
                    R[:],
                    pair[:].unsqueeze(2).to_broadcast([128, NS, NG, RC, BP]),
                    mk16[:].to_broadcast([128, NS, NG, RC, BP]),
                    ALU.mult)
                # scatter: tot[pc, g] = sum over edges of check (pc, g)
                for s in range(NS):
                    nc.tensor.matmul(
                        tot_ps[:],
                        wA[:, s, :],
                        R[:, s].rearrange("p g r b -> p (g r b)"),
                        start=(s == 0), stop=(s == NS - 1),
                    )
                # rebuild fp16 hi/lo of the per-check sums for the gather pass
                nc.scalar.activation(tot_f[:], tot_ps[:], AF.Copy)
                nc.vector.tensor_tensor(
                    Lt_f[:], tot_f[:, :, 0, :], tot_f[:, :, 1, :], ALU.add)
                nc.vector.tensor_copy(tbf[:, :, 0, :], Lt_f[:])
                nc.vector.tensor_tensor(
                    tbf[:, :, 1, :], Lt_f[:], tbf[:, :, 0, :], ALU.subtract)
                nc.vector.tensor_copy(tbf[:, :, 2, :], tot_f[:, :, 2, :])
                # gather: U[pe, s, g] = tot[pc(pe,s), g]
                tot_flat = tbf[:].rearrange("p g r b -> p (g r b)")
                for s in range(NS):
                    nc.tensor.matmul(
                        U_ps[:, s, 0:NG * FW], wB[:, s, :], tot_flat,
                        start=True, stop=True,
                    )
                Uv = U_ps[:, :, 0:NG * FW].rearrange(
                    "p s (g r b) -> p s g r b", g=NG, r=RC)
                nc.vector.tensor_tensor(
                    Um[:], Uv, mk[:].to_broadcast([128, NS, NG, RC, BP]), ALU.mult)
                nc.vector.tensor_reduce(
                    G3[:], Um[:].transpose([0, 1, 3, 4, 2]),
                    axis=mybir.AxisListType.X, op=ALU.add)

                # exclusive check sums: sr = Lt - lt, kx = K - b
                G3q = G3[:].rearrange("p (q j) r b -> p q j r b", q=NQ)
                nc.vector.tensor_tensor(S[:], G3q[:, :, :, 0, :], G3q[:, :, :, 1, :],
                                        ALU.add)
                nc.vector.tensor_tensor(dlt[:], S[:], ltq, ALU.subtract)
                nc.vector.tensor_tensor(
                    db[:], G3q[:, :, :, 2, :],
                    pair_b.rearrange("p (q j) b -> p q j b", q=NQ), ALU.subtract)
                nc.scalar.activation(xp[:], dlt[:], AF.Exp)
                # sign of exclusive product: (-1)^kx
                nc.vector.tensor_copy(ki[:], db[:])
                nc.vector.tensor_scalar(kb[:], ki[:], 1, None, ALU.bitwise_and)
                nc.vector.tensor_scalar(sgnx[:], kb[:], -2.0, 1.0, ALU.mult, ALU.add)
                # y = ln(1 + pd) - ln(1 - pd), pd = sgn * (min(xp, 1) - 2e-7)
                # (clamp: fp16 hi/lo transport noise can push xp past 1 +- the
                #  2e-7 guard; the exact product never exceeds 1)
                nc.vector.tensor_scalar(w[:], xp[:], 1.0, -2e-7, ALU.min, ALU.add)
                nc.vector.tensor_tensor(pd[:], w[:], sgnx[:], ALU.mult)
                nc.scalar.activation(L1[:], pd[:], AF.Ln, bias=1.0)
                nc.scalar.activation(L2[:], pd[:], AF.Ln, bias=1.0, scale=-1.0)
                nc.vector.tensor_tensor(yv[:, :, 0:3, :], L1[:], L2[:], ALU.subtract)

                # variable side is local: Vf = llr + sum_j y_j
                nc.vector.tensor_reduce(
                    Vf[:], yv[:].transpose([0, 1, 3, 2]),
                    axis=mybir.AxisListType.X, op=ALU.add)
                if t < n_iters - 1:
                    nc.vector.tensor_tensor(
                        x[:],
                        Vf[:].unsqueeze(2).to_broadcast([128, NQ, 3, BP]),
                        yv[:, :, 0:3, :], ALU.subtract)
                else:
                    nc.scalar.activation(sg[:], Vf[:], AF.Sign)
                    nc.vector.tensor_scalar(dec_f[:], sg[:], -0.5, 0.5,
                                            ALU.mult, ALU.add)
                    nc.vector.tensor_copy(dec_i[:], dec_f[:])
                    nc.sync.dma_start(
                        out=dec_d.ap().rearrange("b (p q) -> p q b", p=128),
                        in_=dec_i[:],
                    )
    nc.compile()
    return nc


_PROGRAM_CACHE = {}


def _get_program(n_iters: int):
    if n_iters not in _PROGRAM_CACHE:
        _PROGRAM_CACHE[n_iters] = _build_program(n_iters)
    return _PROGRAM_CACHE[n_iters]


def _make_in_maps(llr_in, H_sumC_to_V, H_xe_v_sumc_to_y):
    llr = np.ascontiguousarray(np.asarray(llr_in, dtype=np.float32))
    assert llr.shape == (B, N)
    wA, wB, mk = _derive_structure(H_sumC_to_V, H_xe_v_sumc_to_y)
    wA = np.ascontiguousarray(wA.reshape(128, NS * 128))
    wB = np.ascontiguousarray(wB.reshape(128, NS * 128))
    mk = np.ascontiguousarray(mk.reshape(128, NS * NG))
    return [
        {
            "llr": np.ascontiguousarray(llr[c * BP:(c + 1) * BP]),
            "wA": wA,
            "wB": wB,
            "mk": mk,
        }
        for c in range(NCORES)
    ]


def kernel(llr_in, H_x_to_xe0, H_sumC_to_V, H_sumV_to_C, H_xe_v_sumc_to_y,
           bp_iter_num, **_unused):
    from concourse.bass_utils import run_bass_kernel_spmd

    n_iters = int(np.asarray(bp_iter_num))
    nc = _get_program(n_iters)
    in_maps = _make_in_maps(llr_in, H_sumC_to_V, H_xe_v_sumc_to_y)
    res = run_bass_kernel_spmd(nc, in_maps, list(range(NCORES)))
    out = np.concatenate([res.results[c]["dec"] for c in range(NCORES)], axis=0)
    return out.astype(np.int32)


# revision 19
# speedup vs baseline: 9.9338x; 1.1182x over previous
"""BP LDPC decoder as an on-chip PE-routed Trainium2 kernel.

The reference multiplies dense [E,E] (E=3456) "exclusive sum" operators every
iteration.  Those operators are the check-node / variable-node exclusive sums
of a (DV=3)-regular LDPC graph.  The variable-node side is local to a
partition-major edge layout; the check-node side (a fixed graph permutation +
segmented reduce) is done entirely on the tensor engine with 0/1 routing
matrices held in SBUF:

  scatter: tot[pc, g, :] = sum_s A_s^T @ (pair[:, s] masked into its g slot)
  gather:  U[pe, s, g, :] = B_s^T @ tot ; G = sum_g U * mask

so nothing round-trips through DRAM inside the iteration loop (the baseline
did 54 indirect DMAs per iteration through HBM; that was ~90% of its time).

The routed channels are fp16 (fp32 matmuls run 2 PE passes; fp16 runs one and
gets fast weight load).  The log-magnitude channel is carried as an fp16
hi/lo pair (lo = lt - fp16(lt)), so the reconstructed check sums match fp32
to ~2^-22 relative -- the decoded bits stay bit-exact vs the fp32 reference.
The sign-count channel is exact (small integers).

Sharding: pure data parallel over the batch (16 -> 2 samples per core); the
graph structure (routing matrices) is replicated.
"""

import sys
import numpy as np

for _p in ("/opt/trn_rl_repo", "/root/.axon_site/_ro/trn_rl_repo"):
    if _p not in sys.path:
        sys.path.insert(0, _p)

N = 1152          # variables
E = 3456          # edges (DV=3 per variable)
B = 16            # batch
NCORES = 8
BP = B // NCORES  # batch per core
NQ = N // 128     # variables per partition (9)
NS = E // 128     # edge slots per partition (27)
NG = 5            # check groups of 128 (576 checks -> 5*128 slots)
RC = 3            # routed channels: (lt_hi, lt_lo, b)
FW = RC * BP      # matmul free width per (s, g)

_A32 = float(np.float32(1.0 + 1e-8))
_B32 = float(np.float32(1.0 - 1e-8))


def _derive_structure(H_sumC_to_V, H_xe_v_sumc_to_y):
    """Recover the LDPC graph and build the PE routing operators.

    Device edge order is col-major (variable-major): e = 3*v + j, variable v
    on partition v // NQ, slot s = e % NS.  Check c sits at PSUM row c % 128
    in group c // 128.
    Returns:
      wA [128, NS, 128] f16: wA[pe, s, pc] = 1 iff edge (pe,s) -> check row pc
      wB [128, NS, 128] f16: wB[pc, s, pe] = wA[pe, s, pc]   (gather routing)
      mk [128, NS, NG] f32:  mk[pe, s, g]  = 1 iff edge (pe,s) -> group g
    """
    H_sumC_to_V = np.asarray(H_sumC_to_V)
    H_xe_v_sumc_to_y = np.asarray(H_xe_v_sumc_to_y)
    cols_rm = np.argmax(H_xe_v_sumc_to_y, axis=0)        # variable of row-major edge
    p_r2l = np.argsort(cols_rm, kind="stable")           # col-major pos -> row-major idx
    p_l2r = np.argsort(p_r2l)
    Br = H_sumC_to_V[:, p_l2r]                           # same-check adjacency (row-major)
    same = Br[np.arange(E - 1), np.arange(1, E)] > 0
    check_id = np.concatenate([[0], np.cumsum(~same)]).astype(np.int64)
    ck = check_id[p_r2l]                                 # check of device edge e
    assert ck.max() < NG * 128
    pc = (ck % 128).astype(np.int64).reshape(128, NS)    # [pe, s]
    g = (ck // 128).astype(np.int64).reshape(128, NS)
    pe_idx = np.arange(128)[:, None].repeat(NS, 1)
    s_idx = np.arange(NS)[None, :].repeat(128, 0)
    wA = np.zeros((128, NS, 128), np.float16)
    wA[pe_idx, s_idx, pc] = 1.0
    wB = np.zeros((128, NS, 128), np.float16)
    wB[pc, s_idx, pe_idx] = 1.0
    mk = np.zeros((128, NS, 1, NG), np.float32)
    mk[pe_idx, s_idx, 0, g] = 1.0
    mkb = np.zeros((128, NS, NG, RC, BP), np.float16)
    mkb[pe_idx, s_idx, g] = 1.0
    return wA, wB, mk, mkb


def _build_program(n_iters: int):
    import concourse.bacc as bacc
    import concourse.hw_specs as hw_specs
    # Force every activation onto the one table set containing all our
    # functions (ln/exp/sign/abs); otherwise the chooser ping-pongs between
    # sets and reloads tables (~2.7us per reload).
    if not getattr(bacc, "_act_tables_pinned", False):
        _orig_get = hw_specs.get_activation_tables

        def _pinned(arch):
            tabs = _orig_get(arch)
            keep = "natural_log_exp_and_others"
            if keep in tabs:
                tabs = {k: (v if k == keep else set()) for k, v in tabs.items()}
            return tabs

        bacc.get_activation_tables = _pinned
        bacc._act_tables_pinned = True
    import concourse.mybir as mybir
    import concourse.tile as tile

    f32 = mybir.dt.float32
    f16 = mybir.dt.float16
    i32 = mybir.dt.int32
    AF = mybir.ActivationFunctionType
    ALU = mybir.AluOpType

    nc = bacc.Bacc("TRN2", target_bir_lowering=False, debug=False)

    llr_d = nc.declare_dram_parameter("llr", [BP, N], f32, isOutput=False)
    wA_d = nc.declare_dram_parameter("wA", [128, NS * 128], f16, isOutput=False)
    wB_d = nc.declare_dram_parameter("wB", [128, NS * 128], f16, isOutput=False)
    mk_d = nc.declare_dram_parameter("mk", [128, NS * NG], f32, isOutput=False)
    mkb_d = nc.declare_dram_parameter("mkb", [128, NS * NG * RC * BP], f16,
                                      isOutput=False)
    dec_d = nc.declare_dram_parameter("dec", [BP, N], i32, isOutput=True)

    with tile.TileContext(nc) as tc:
        with tc.tile_pool(name="st", bufs=1) as st, \
             tc.tile_pool(name="ps", bufs=1, space="PSUM") as ps:
            wA = st.tile([128, NS, 128], f16)
            wB = st.tile([128, NS, 128], f16)
            mk = st.tile([128, NS, 1, NG], f32)
            mkb = st.tile([128, NS, NG, RC, BP], f16)  # pre-broadcast fp16 mask
            llr_sb = st.tile([128, NQ, BP], f32)
            x = st.tile([128, NQ, 3, BP], f32)         # per-edge v->c messages
            lt_f = st.tile([128, NS, BP], f32)         # fp32 log-magnitude
            pair = st.tile([128, NS, RC, BP], f16)     # (lt_hi, lt_lo, b) fp16
            R = st.tile([128, NS, NG, RC, BP], f16)    # mask-expanded rhs
            tbf = st.tile([128, RC, BP, NG], f16)      # gather rhs (hi, lo, K)
            Lt_f = st.tile([128, NG, BP], f32)
            Um = st.tile([128, NS, RC, BP, NG], f32)
            G3 = st.tile([128, NS, RC, BP], f32)       # gathered (hi, lo, K) sums
            S = st.tile([128, NQ, 3, BP], f32)
            dlt = st.tile([128, NQ, 3, BP], f32)
            db = st.tile([128, NQ, 3, BP], f32)
            ax = st.tile([128, NQ, 3, BP], f32)
            u = st.tile([128, NQ, 3, BP], f32)
            lnum = st.tile([128, NQ, 3, BP], f32)
            lden = st.tile([128, NQ, 3, BP], f32)
            xp = st.tile([128, NQ, 3, BP], f32)
            ki = st.tile([128, NQ, 3, BP], i32)
            kb = st.tile([128, NQ, 3, BP], i32)
            sgnx = st.tile([128, NQ, 3, BP], f32)
            w = st.tile([128, NQ, 3, BP], f32)
            pd = st.tile([128, NQ, 3, BP], f32)
            L1 = st.tile([128, NQ, 3, BP], f32)
            L2 = st.tile([128, NQ, 3, BP], f32)
            yv = st.tile([128, NQ, 4, BP], f32)        # slots 0..2 = y_j, 3 = llr
            Vf = st.tile([128, NQ, BP], f32)           # llr + sum_j y_j
            sg = st.tile([128, NQ, BP], f32)
            dec_f = st.tile([128, NQ, BP], f32)
            dec_i = st.tile([128, NQ, BP], i32)

            tot_ps = ps.tile([128, NG, RC, BP], f32)   # 120B -> 1 bank
            U_ps = ps.tile([128, NS, 32], f32)         # 32-f32 stride: bank-aligned

            # ---- init (llr first: iteration 0's front chain only needs it) ----
            nc.sync.dma_start(
                out=llr_sb[:], in_=llr_d.ap().rearrange("b (p q) -> p q b", p=128)
            )
            nc.sync.dma_start(out=wA[:], in_=wA_d.ap())
            nc.sync.dma_start(out=mkb[:], in_=mkb_d.ap())
            nc.sync.dma_start(out=mk[:], in_=mk_d.ap())
            nc.sync.dma_start(out=wB[:], in_=wB_d.ap())
            nc.vector.tensor_copy(
                x[:], llr_sb[:].unsqueeze(2).to_broadcast([128, NQ, 3, BP])
            )
            nc.vector.tensor_copy(yv[:, :, 3, :], llr_sb[:])

            pair_hi = pair[:, :, 0, :]
            pair_lo = pair[:, :, 1, :]
            pair_b = pair[:, :, 2, :]
            ltq = lt_f[:].rearrange("p (q j) b -> p q j b", q=NQ)

            for t in range(n_iters):
                # lt = ln(1e-8 + tanh(|x|/2)) computed exp/ln-only:
                #   u = exp(-|x|); lt = ln(A - B*u) - ln(1 + u)
                nc.scalar.activation(ax[:], x[:], AF.Abs)
                nc.scalar.activation(u[:], ax[:], AF.Exp, scale=-1.0)
                nc.scalar.activation(lnum[:], u[:], AF.Ln, bias=_A32, scale=-_B32)
                nc.scalar.activation(lden[:], u[:], AF.Ln, bias=1.0)
                nc.vector.tensor_tensor(ltq, lnum[:], lden[:], ALU.subtract)
                # fp16 hi/lo split of lt; b = 1 if x < 0 else 0
                nc.vector.tensor_copy(pair_hi, lt_f[:])
                nc.vector.tensor_tensor(pair_lo, lt_f[:], pair_hi, ALU.subtract)
                nc.vector.tensor_scalar(
                    pair_b.rearrange("p (q j) b -> p q j b", q=NQ),
                    x[:], 0.0, None, ALU.is_lt)

                # mask-expand the per-edge rows into their check-group slot
                # (two halves so the scatter matmuls start on half A early)
                for sl, sh in ((0, 14), (14, NS)):
                    nc.vector.tensor_tensor(
                        R[:, sl:sh],
                        pair[:, sl:sh].unsqueeze(2).to_broadcast(
                            [128, sh - sl, NG, RC, BP]),
                        mkb[:, sl:sh],
                        ALU.mult)
                # scatter: tot[pc, g] = sum over edges of check (pc, g)
                for s in range(NS):
                    nc.tensor.matmul(
                        tot_ps[:],
                        wA[:, s, :],
                        R[:, s].rearrange("p g r b -> p (g r b)"),
                        start=(s == 0), stop=(s == NS - 1),
                    )
                # rebuild fp16 hi/lo of the per-check sums for the gather pass
                nc.vector.tensor_reduce(
                    Lt_f[:], tot_ps[:, :, 0:2, :].transpose([0, 1, 3, 2]),
                    axis=mybir.AxisListType.X, op=ALU.add)
                nc.vector.tensor_copy(
                    tbf[:, 0, :, :], Lt_f[:].transpose([0, 2, 1]))
                nc.vector.tensor_tensor(
                    tbf[:, 1, :, :], Lt_f[:].transpose([0, 2, 1]),
                    tbf[:, 0, :, :], ALU.subtract)
                nc.vector.tensor_copy(
                    tbf[:, 2, :, :], tot_ps[:, :, 2, :].transpose([0, 2, 1]))
                # gather: U[pe, s, g] = tot[pc(pe,s), g]  (free order r, b, g)
                tot_flat = tbf[:].rearrange("p r b g -> p (r b g)")
                for s in range(NS):
                    nc.tensor.matmul(
                        U_ps[:, s, 0:NG * FW], wB[:, s, :], tot_flat,
                        start=True, stop=True,
                    )
                Uv = U_ps[:, :, 0:NG * FW].rearrange(
                    "p s (rb g) -> p s rb g", g=NG)
                Um4 = Um[:].rearrange("p s r b g -> p s (r b) g")
                # mask-select in two halves (overlaps the tail of the gather)
                nc.vector.tensor_tensor(
                    Um4[:, 0:16], Uv[:, 0:16],
                    mk[:, 0:16].to_broadcast([128, 16, RC * BP, NG]), ALU.mult)
                nc.vector.tensor_tensor(
                    Um4[:, 16:NS], Uv[:, 16:NS],
                    mk[:, 16:NS].to_broadcast([128, NS - 16, RC * BP, NG]), ALU.mult)
                nc.vector.tensor_reduce(
                    G3[:], Um4, axis=mybir.AxisListType.X, op=ALU.add)

                # exclusive check sums: sr = Lt - lt, kx = K - b
                G3q = G3[:].rearrange("p (q j) r b -> p q j r b", q=NQ)
                nc.vector.tensor_tensor(S[:], G3q[:, :, :, 0, :], G3q[:, :, :, 1, :],
                                        ALU.add)
                nc.vector.tensor_tensor(dlt[:], S[:], ltq, ALU.subtract)
                nc.vector.tensor_tensor(
                    db[:], G3q[:, :, :, 2, :],
                    pair_b.rearrange("p (q j) b -> p q j b", q=NQ), ALU.subtract)
                nc.scalar.activation(xp[:], dlt[:], AF.Exp)
                # sign of exclusive product: (-1)^kx
                nc.vector.tensor_copy(ki[:], db[:])
                nc.vector.tensor_scalar(kb[:], ki[:], 1, None, ALU.bitwise_and)
                nc.vector.tensor_scalar(sgnx[:], kb[:], -2.0, 1.0, ALU.mult, ALU.add)
                # y = ln(1 + pd) - ln(1 - pd), pd = sgn * (min(xp, 1) - 2e-7)
                # (clamp: fp16 hi/lo transport noise can push xp past 1 +- the
                #  2e-7 guard; the exact product never exceeds 1)
                nc.vector.tensor_scalar(w[:], xp[:], 1.0, -2e-7, ALU.min, ALU.add)
                nc.vector.tensor_tensor(pd[:], w[:], sgnx[:], ALU.mult)
                nc.scalar.activation(L1[:], pd[:], AF.Ln, bias=1.0)
                nc.scalar.activation(L2[:], pd[:], AF.Ln, bias=1.0, scale=-1.0)
                nc.vector.tensor_tensor(yv[:, :, 0:3, :], L1[:], L2[:], ALU.subtract)

                # variable side is local: Vf = llr + sum_j y_j
                nc.vector.tensor_reduce(
                    Vf[:], yv[:].transpose([0, 1, 3, 2]),
                    axis=mybir.AxisListType.X, op=ALU.add)
                if t < n_iters - 1:
                    nc.vector.tensor_tensor(
                        x[:],
                        Vf[:].unsqueeze(2).to_broadcast([128, NQ, 3, BP]),
                        yv[:, :, 0:3, :], ALU.subtract)
                else:
                    nc.scalar.activation(sg[:], Vf[:], AF.Sign)
                    nc.vector.tensor_scalar(dec_f[:], sg[:], -0.5, 0.5,
                                            ALU.mult, ALU.add)
                    nc.vector.tensor_copy(dec_i[:], dec_f[:])
                    nc.sync.dma_start(
                        out=dec_d.ap().rearrange("b (p q) -> p q b", p=128),
                        in_=dec_i[:],
                    )
    nc.compile()
    return nc


_PROGRAM_CACHE = {}


def _get_program(n_iters: int):
    if n_iters not in _PROGRAM_CACHE:
        _PROGRAM_CACHE[n_iters] = _build_program(n_iters)
    return _PROGRAM_CACHE[n_iters]


def _make_in_maps(llr_in, H_sumC_to_V, H_xe_v_sumc_to_y):
    llr = np.ascontiguousarray(np.asarray(llr_in, dtype=np.float32))
    assert llr.shape == (B, N)
    wA, wB, mk, mkb = _derive_structure(H_sumC_to_V, H_xe_v_sumc_to_y)
    wA = np.ascontiguousarray(wA.reshape(128, NS * 128))
    wB = np.ascontiguousarray(wB.reshape(128, NS * 128))
    mk = np.ascontiguousarray(mk.reshape(128, NS * NG))
    mkb = np.ascontiguousarray(mkb.reshape(128, NS * NG * RC * BP))
    return [
        {
            "llr": np.ascontiguousarray(llr[c * BP:(c + 1) * BP]),
            "wA": wA,
            "wB": wB,
            "mk": mk,
            "mkb": mkb,
        }
        for c in range(NCORES)
    ]


def kernel(llr_in, H_x_to_xe0, H_sumC_to_V, H_sumV_to_C, H_xe_v_sumc_to_y,
           bp_iter_num, **_unused):
    from concourse.bass_utils import run_bass_kernel_spmd

    n_iters = int(np.asarray(bp_iter_num))
    nc = _get_program(n_iters)
    in_maps = _make_in_maps(llr_in, H_sumC_to_V, H_xe_v_sumc_to_y)
    res = run_bass_kernel_spmd(nc, in_maps, list(range(NCORES)))
    out = np.concatenate([res.results[c]["dec"] for c in range(NCORES)], axis=0)
    return out.astype(np.int32)


# revision 23
# speedup vs baseline: 9.9386x; 1.0005x over previous
"""BP LDPC decoder as an on-chip PE-routed Trainium2 kernel.

The reference multiplies dense [E,E] (E=3456) "exclusive sum" operators every
iteration.  Those operators are the check-node / variable-node exclusive sums
of a (DV=3)-regular LDPC graph.  The variable-node side is local to a
partition-major edge layout; the check-node side (a fixed graph permutation +
segmented reduce) is done entirely on the tensor engine with 0/1 routing
matrices held in SBUF:

  scatter: tot[pc, g, :] = sum_s A_s^T @ (pair[:, s] masked into its g slot)
  gather:  U[pe, s, g, :] = B_s^T @ tot ; G = sum_g U * mask

so nothing round-trips through DRAM inside the iteration loop (the baseline
did 54 indirect DMAs per iteration through HBM; that was ~90% of its time).

The routed channels are fp16 (fp32 matmuls run 2 PE passes; fp16 runs one and
gets fast weight load).  The log-magnitude channel is carried as an fp16
hi/lo pair (lo = lt - fp16(lt)), so the reconstructed check sums match fp32
to ~2^-22 relative -- the decoded bits stay bit-exact vs the fp32 reference.
The sign-count channel is exact (small integers).

Sharding: pure data parallel over the batch (16 -> 2 samples per core); the
graph structure (routing matrices) is replicated.
"""

import sys
import numpy as np

for _p in ("/opt/trn_rl_repo", "/root/.axon_site/_ro/trn_rl_repo"):
    if _p not in sys.path:
        sys.path.insert(0, _p)

N = 1152          # variables
E = 3456          # edges (DV=3 per variable)
B = 16            # batch
NCORES = 8
BP = B // NCORES  # batch per core
NQ = N // 128     # variables per partition (9)
NS = E // 128     # edge slots per partition (27)
NG = 5            # check groups of 128 (576 checks -> 5*128 slots)
RC = 3            # routed channels: (lt_hi, lt_lo, b)
FW = RC * BP      # matmul free width per (s, g)

_A32 = float(np.float32(1.0 + 1e-8))
_B32 = float(np.float32(1.0 - 1e-8))


def _derive_structure(H_sumC_to_V, H_xe_v_sumc_to_y):
    """Recover the LDPC graph and build the PE routing operators.

    Device edge order is col-major (variable-major): e = 3*v + j, variable v
    on partition v // NQ, slot s = e % NS.  Check c sits at PSUM row c % 128
    in group c // 128.
    Returns:
      wA [128, NS, 128] f16: wA[pe, s, pc] = 1 iff edge (pe,s) -> check row pc
      wB [128, NS, 128] f16: wB[pc, s, pe] = wA[pe, s, pc]   (gather routing)
      mk [128, NS, NG] f32:  mk[pe, s, g]  = 1 iff edge (pe,s) -> group g
    """
    H_sumC_to_V = np.asarray(H_sumC_to_V)
    H_xe_v_sumc_to_y = np.asarray(H_xe_v_sumc_to_y)
    cols_rm = np.argmax(H_xe_v_sumc_to_y, axis=0)        # variable of row-major edge
    p_r2l = np.argsort(cols_rm, kind="stable")           # col-major pos -> row-major idx
    p_l2r = np.argsort(p_r2l)
    Br = H_sumC_to_V[:, p_l2r]                           # same-check adjacency (row-major)
    same = Br[np.arange(E - 1), np.arange(1, E)] > 0
    check_id = np.concatenate([[0], np.cumsum(~same)]).astype(np.int64)
    ck = check_id[p_r2l]                                 # check of device edge e
    assert ck.max() < NG * 128
    pc = (ck % 128).astype(np.int64).reshape(128, NS)    # [pe, s]
    g = (ck // 128).astype(np.int64).reshape(128, NS)
    pe_idx = np.arange(128)[:, None].repeat(NS, 1)
    s_idx = np.arange(NS)[None, :].repeat(128, 0)
    wA = np.zeros((128, NS, 128), np.float16)
    wA[pe_idx, s_idx, pc] = 1.0
    wB = np.zeros((128, NS, 128), np.float16)
    wB[pc, s_idx, pe_idx] = 1.0
    mk = np.zeros((128, NS, 1, NG), np.float32)
    mk[pe_idx, s_idx, 0, g] = 1.0
    mkb = np.zeros((128, NS, NG, RC, BP), np.float16)
    mkb[pe_idx, s_idx, g] = 1.0
    return wA, wB, mk, mkb


def _build_program(n_iters: int):
    import concourse.bacc as bacc
    import concourse.hw_specs as hw_specs
    # Force every activation onto the one table set containing all our
    # functions (ln/exp/sign/abs); otherwise the chooser ping-pongs between
    # sets and reloads tables (~2.7us per reload).
    if not getattr(bacc, "_act_tables_pinned", False):
        _orig_get = hw_specs.get_activation_tables

        def _pinned(arch):
            tabs = _orig_get(arch)
            keep = "natural_log_exp_and_others"
            if keep in tabs:
                tabs = {k: (v if k == keep else set()) for k, v in tabs.items()}
            return tabs

        bacc.get_activation_tables = _pinned
        bacc._act_tables_pinned = True
    import concourse.mybir as mybir
    import concourse.tile as tile

    f32 = mybir.dt.float32
    f16 = mybir.dt.float16
    i32 = mybir.dt.int32
    AF = mybir.ActivationFunctionType
    ALU = mybir.AluOpType

    nc = bacc.Bacc("TRN2", target_bir_lowering=False, debug=False)

    llr_d = nc.declare_dram_parameter("llr", [BP, N], f32, isOutput=False)
    wA_d = nc.declare_dram_parameter("wA", [128, NS * 128], f16, isOutput=False)
    wB_d = nc.declare_dram_parameter("wB", [128, NS * 128], f16, isOutput=False)
    mkb_d = nc.declare_dram_parameter("mkb", [128, NS * NG * RC * BP], f16,
                                      isOutput=False)
    dec_d = nc.declare_dram_parameter("dec", [BP, N], i32, isOutput=True)

    with tile.TileContext(nc) as tc:
        with tc.tile_pool(name="st", bufs=1) as st, \
             tc.tile_pool(name="ps", bufs=1, space="PSUM") as ps:
            wA = st.tile([128, NS, 128], f16)
            wB = st.tile([128, NS, 128], f16)
            mk = st.tile([128, NS, 1, NG], f32)
            mkb = st.tile([128, NS, NG, RC, BP], f16)  # pre-broadcast fp16 mask
            llr_sb = st.tile([128, NQ, BP], f32)
            x = st.tile([128, NQ, 3, BP], f32)         # per-edge v->c messages
            lt_f = st.tile([128, NS, BP], f32)         # fp32 log-magnitude
            pair = st.tile([128, NS, RC, BP], f16)     # (lt_hi, lt_lo, b) fp16
            R = st.tile([128, NS, NG, RC, BP], f16)    # mask-expanded rhs
            tbf = st.tile([128, RC, BP, NG], f16)      # gather rhs (hi, lo, K)
            Lt_f = st.tile([128, NG, BP], f32)
            Um = st.tile([128, NS, RC, BP, NG], f32)
            G3 = st.tile([128, NS, RC, BP], f32)       # gathered (hi, lo, K) sums
            S = st.tile([128, NQ, 3, BP], f32)
            dlt = st.tile([128, NQ, 3, BP], f32)
            db = st.tile([128, NQ, 3, BP], f32)
            ax = st.tile([128, NQ, 3, BP], f32)
            u = st.tile([128, NQ, 3, BP], f32)
            lnum = st.tile([128, NQ, 3, BP], f32)
            lden = st.tile([128, NQ, 3, BP], f32)
            xp = st.tile([128, NQ, 3, BP], f32)
            ki = st.tile([128, NQ, 3, BP], i32)
            kb = st.tile([128, NQ, 3, BP], i32)
            sgnx = st.tile([128, NQ, 3, BP], f32)
            w = st.tile([128, NQ, 3, BP], f32)
            pd = st.tile([128, NQ, 3, BP], f32)
            L1 = st.tile([128, NQ, 3, BP], f32)
            L2 = st.tile([128, NQ, 3, BP], f32)
            yv = st.tile([128, NQ, 4, BP], f32)        # slots 0..2 = y_j, 3 = llr
            Vf = st.tile([128, NQ, BP], f32)           # llr + sum_j y_j
            sg = st.tile([128, NQ, BP], f32)
            dec_f = st.tile([128, NQ, BP], f32)
            dec_i = st.tile([128, NQ, BP], i32)

            tot_ps = ps.tile([128, NG, RC, BP], f32)   # 120B -> 1 bank
            U_ps = ps.tile([128, NS, 32], f32)         # 32-f32 stride: bank-aligned

            # ---- init (ordered by when iteration 0 needs each tensor) ----
            nc.sync.dma_start(
                out=llr_sb[:], in_=llr_d.ap().rearrange("b (p q) -> p q b", p=128)
            )
            wA_ap = wA_d.ap().rearrange("p (s c) -> p s c", s=NS)
            wB_ap = wB_d.ap().rearrange("p (s c) -> p s c", s=NS)
            nc.sync.dma_start(out=mkb[:], in_=mkb_d.ap())
            nc.sync.dma_start(out=wA[:, 0:14], in_=wA_ap[:, 0:14])
            nc.sync.dma_start(out=wA[:, 14:NS], in_=wA_ap[:, 14:NS])
            nc.sync.dma_start(out=wB[:, 0:14], in_=wB_ap[:, 0:14])
            nc.sync.dma_start(out=wB[:, 14:NS], in_=wB_ap[:, 14:NS])
            # f32 mask for the PSUM-side select, derived from the fp16 one
            nc.vector.tensor_copy(mk[:, :, 0, :], mkb[:, :, :, 0, 0])
            nc.vector.tensor_copy(
                x[:], llr_sb[:].unsqueeze(2).to_broadcast([128, NQ, 3, BP])
            )
            nc.vector.tensor_copy(yv[:, :, 3, :], llr_sb[:])

            pair_hi = pair[:, :, 0, :]
            pair_lo = pair[:, :, 1, :]
            pair_b = pair[:, :, 2, :]
            ltq = lt_f[:].rearrange("p (q j) b -> p q j b", q=NQ)

            for t in range(n_iters):
                # lt = ln(1e-8 + tanh(|x|/2)) computed exp/ln-only:
                #   u = exp(-|x|); lt = ln(A - B*u) - ln(1 + u)
                nc.scalar.activation(ax[:], x[:], AF.Abs)
                nc.scalar.activation(u[:], ax[:], AF.Exp, scale=-1.0)
                nc.scalar.activation(lnum[:], u[:], AF.Ln, bias=_A32, scale=-_B32)
                nc.scalar.activation(lden[:], u[:], AF.Ln, bias=1.0)
                nc.vector.tensor_tensor(ltq, lnum[:], lden[:], ALU.subtract)
                # fp16 hi/lo split of lt; b = 1 if x < 0 else 0
                nc.vector.tensor_copy(pair_hi, lt_f[:])
                nc.vector.tensor_tensor(pair_lo, lt_f[:], pair_hi, ALU.subtract)
                nc.vector.tensor_scalar(
                    pair_b.rearrange("p (q j) b -> p q j b", q=NQ),
                    x[:], 0.0, None, ALU.is_lt)

                # mask-expand the per-edge rows into their check-group slot
                # (two halves so the scatter matmuls start on half A early)
                for sl, sh in ((0, 14), (14, NS)):
                    nc.vector.tensor_tensor(
                        R[:, sl:sh],
                        pair[:, sl:sh].unsqueeze(2).to_broadcast(
                            [128, sh - sl, NG, RC, BP]),
                        mkb[:, sl:sh],
                        ALU.mult)
                # scatter: tot[pc, g] = sum over edges of check (pc, g)
                for s in range(NS):
                    nc.tensor.matmul(
                        tot_ps[:],
                        wA[:, s, :],
                        R[:, s].rearrange("p g r b -> p (g r b)"),
                        start=(s == 0), stop=(s == NS - 1),
                    )
                # rebuild fp16 hi/lo of the per-check sums for the gather pass
                nc.vector.tensor_reduce(
                    Lt_f[:], tot_ps[:, :, 0:2, :].transpose([0, 1, 3, 2]),
                    axis=mybir.AxisListType.X, op=ALU.add)
                nc.vector.tensor_copy(
                    tbf[:, 0, :, :], Lt_f[:].transpose([0, 2, 1]))
                nc.vector.tensor_tensor(
                    tbf[:, 1, :, :], Lt_f[:].transpose([0, 2, 1]),
                    tbf[:, 0, :, :], ALU.subtract)
                nc.vector.tensor_copy(
                    tbf[:, 2, :, :], tot_ps[:, :, 2, :].transpose([0, 2, 1]))
                # gather: U[pe, s, g] = tot[pc(pe,s), g]  (free order r, b, g)
                tot_flat = tbf[:].rearrange("p r b g -> p (r b g)")
                for s in range(NS):
                    nc.tensor.matmul(
                        U_ps[:, s, 0:NG * FW], wB[:, s, :], tot_flat,
                        start=True, stop=True,
                    )
                Uv = U_ps[:, :, 0:NG * FW].rearrange(
                    "p s (rb g) -> p s rb g", g=NG)
                Um4 = Um[:].rearrange("p s r b g -> p s (r b) g")
                nc.vector.tensor_tensor(
                    Um4, Uv,
                    mk[:].to_broadcast([128, NS, RC * BP, NG]), ALU.mult)
                nc.vector.tensor_reduce(
                    G3[:], Um4, axis=mybir.AxisListType.X, op=ALU.add)

                # exclusive check sums: sr = Lt - lt, kx = K - b
                G3q = G3[:].rearrange("p (q j) r b -> p q j r b", q=NQ)
                nc.vector.tensor_tensor(S[:], G3q[:, :, :, 0, :], G3q[:, :, :, 1, :],
                                        ALU.add)
                nc.vector.tensor_tensor(dlt[:], S[:], ltq, ALU.subtract)
                nc.vector.tensor_tensor(
                    db[:], G3q[:, :, :, 2, :],
                    pair_b.rearrange("p (q j) b -> p q j b", q=NQ), ALU.subtract)
                nc.scalar.activation(xp[:], dlt[:], AF.Exp)
                # sign of exclusive product: (-1)^kx
                nc.vector.tensor_copy(ki[:], db[:])
                nc.vector.tensor_scalar(kb[:], ki[:], 1, None, ALU.bitwise_and)
                nc.vector.tensor_scalar(sgnx[:], kb[:], -2.0, 1.0, ALU.mult, ALU.add)
                # y = ln(1 + pd) - ln(1 - pd), pd = sgn * (min(xp, 1) - 2e-7)
                # (clamp: fp16 hi/lo transport noise can push xp past 1 +- the
                #  2e-7 guard; the exact product never exceeds 1)
                nc.vector.tensor_scalar(w[:], xp[:], 1.0, -2e-7, ALU.min, ALU.add)
                nc.vector.tensor_tensor(pd[:], w[:], sgnx[:], ALU.mult)
                nc.scalar.activation(L1[:], pd[:], AF.Ln, bias=1.0)
                nc.scalar.activation(L2[:], pd[:], AF.Ln, bias=1.0, scale=-1.0)
                nc.vector.tensor_tensor(yv[:, :, 0:3, :], L1[:], L2[:], ALU.subtract)

                # variable side is local: Vf = llr + sum_j y_j
                nc.vector.tensor_reduce(
                    Vf[:], yv[:].transpose([0, 1, 3, 2]),
                    axis=mybir.AxisListType.X, op=ALU.add)
                if t < n_iters - 1:
                    nc.vector.tensor_tensor(
                        x[:],
                        Vf[:].unsqueeze(2).to_broadcast([128, NQ, 3, BP]),
                        yv[:, :, 0:3, :], ALU.subtract)
                else:
                    nc.scalar.activation(sg[:], Vf[:], AF.Sign)
                    nc.vector.tensor_scalar(dec_f[:], sg[:], -0.5, 0.5,
                                            ALU.mult, ALU.add)
                    nc.vector.tensor_copy(dec_i[:], dec_f[:])
                    nc.sync.dma_start(
                        out=dec_d.ap().rearrange("b (p q) -> p q b", p=128),
                        in_=dec_i[:],
                    )
    nc.compile()
    return nc


_PROGRAM_CACHE = {}


def _get_program(n_iters: int):
    if n_iters not in _PROGRAM_CACHE:
        _PROGRAM_CACHE[n_iters] = _build_program(n_iters)
    return _PROGRAM_CACHE[n_iters]


def _make_in_maps(llr_in, H_sumC_to_V, H_xe_v_sumc_to_y):
    llr = np.ascontiguousarray(np.asarray(llr_in, dtype=np.float32))
    assert llr.shape == (B, N)
    wA, wB, mk, mkb = _derive_structure(H_sumC_to_V, H_xe_v_sumc_to_y)
    wA = np.ascontiguousarray(wA.reshape(128, NS * 128))
    wB = np.ascontiguousarray(wB.reshape(128, NS * 128))
    mkb = np.ascontiguousarray(mkb.reshape(128, NS * NG * RC * BP))
    return [
        {
            "llr": np.ascontiguousarray(llr[c * BP:(c + 1) * BP]),
            "wA": wA,
            "wB": wB,
            "mkb": mkb,
        }
        for c in range(NCORES)
    ]


def kernel(llr_in, H_x_to_xe0, H_sumC_to_V, H_sumV_to_C, H_xe_v_sumc_to_y,
           bp_iter_num, **_unused):
    from concourse.bass_utils import run_bass_kernel_spmd

    n_iters = int(np.asarray(bp_iter_num))
    nc = _get_program(n_iters)
    in_maps = _make_in_maps(llr_in, H_sumC_to_V, H_xe_v_sumc_to_y)
    res = run_bass_kernel_spmd(nc, in_maps, list(range(NCORES)))
    out = np.concatenate([res.results[c]["dec"] for c in range(NCORES)], axis=0)
    return out.astype(np.int32)


# revision 27
# speedup vs baseline: 10.4659x; 1.0531x over previous
"""BP LDPC decoder as an on-chip PE-routed Trainium2 kernel.

The reference multiplies dense [E,E] (E=3456) "exclusive sum" operators every
iteration.  Those operators are the check-node / variable-node exclusive sums
of a (DV=3)-regular LDPC graph.  The variable-node side is local to a
partition-major edge layout; the check-node side (a fixed graph permutation +
segmented reduce) is done entirely on the tensor engine with 0/1 routing
matrices held in SBUF:

  scatter: tot[pc, g, :] = sum_s A_s^T @ (pair[:, s] masked into its g slot)
  gather:  U[pe, s, g, :] = B_s^T @ tot ; G = sum_g U * mask

so nothing round-trips through DRAM inside the iteration loop (the baseline
did 54 indirect DMAs per iteration through HBM; that was ~90% of its time).

The routed channels are fp16 (fp32 matmuls run 2 PE passes; fp16 runs one and
gets fast weight load).  The log-magnitude channel is carried as an fp16
hi/lo pair (lo = lt - fp16(lt)), so the reconstructed check sums match fp32
to ~2^-22 relative -- the decoded bits stay bit-exact vs the fp32 reference.
The sign-count channel is exact (small integers).

Sharding: pure data parallel over the batch (16 -> 2 samples per core); the
graph structure (routing matrices) is replicated.
"""

import sys
import numpy as np

for _p in ("/opt/trn_rl_repo", "/root/.axon_site/_ro/trn_rl_repo"):
    if _p not in sys.path:
        sys.path.insert(0, _p)

N = 1152          # variables
E = 3456          # edges (DV=3 per variable)
B = 16            # batch
NCORES = 8
BP = B // NCORES  # batch per core
NQ = N // 128     # variables per partition (9)
NS = E // 128     # edge slots per partition (27)
NG = 5            # check groups of 128 (576 checks -> 5*128 slots)
RC = 3            # routed channels: (lt_hi, lt_lo, b)
FW = RC * BP      # matmul free width per (s, g)

_A32 = float(np.float32(1.0 + 1e-8))
_B32 = float(np.float32(1.0 - 1e-8))


def _derive_structure(H_sumC_to_V, H_xe_v_sumc_to_y):
    """Recover the LDPC graph and build the PE routing operators.

    Device edge order is col-major (variable-major): e = 3*v + j, variable v
    on partition v // NQ, slot s = e % NS.  Check c sits at PSUM row c % 128
    in group c // 128.
    Returns:
      wA [128, NS, 128] f16: wA[pe, s, pc] = 1 iff edge (pe,s) -> check row pc
      wB [128, NS, 128] f16: wB[pc, s, pe] = wA[pe, s, pc]   (gather routing)
      mk [128, NS, NG] f32:  mk[pe, s, g]  = 1 iff edge (pe,s) -> group g
    """
    H_sumC_to_V = np.asarray(H_sumC_to_V)
    H_xe_v_sumc_to_y = np.asarray(H_xe_v_sumc_to_y)
    cols_rm = np.argmax(H_xe_v_sumc_to_y, axis=0)        # variable of row-major edge
    p_r2l = np.argsort(cols_rm, kind="stable")           # col-major pos -> row-major idx
    p_l2r = np.argsort(p_r2l)
    Br = H_sumC_to_V[:, p_l2r]                           # same-check adjacency (row-major)
    same = Br[np.arange(E - 1), np.arange(1, E)] > 0
    check_id = np.concatenate([[0], np.cumsum(~same)]).astype(np.int64)
    ck = check_id[p_r2l]                                 # check of device edge e
    assert ck.max() < NG * 128
    pc = (ck % 128).astype(np.int64).reshape(128, NS)    # [pe, s]
    g = (ck // 128).astype(np.int64).reshape(128, NS)
    pe_idx = np.arange(128)[:, None].repeat(NS, 1)
    s_idx = np.arange(NS)[None, :].repeat(128, 0)
    wA = np.zeros((128, NS, 128), np.float16)
    wA[pe_idx, s_idx, pc] = 1.0
    wB = np.zeros((128, NS, 128), np.float16)
    wB[pc, s_idx, pe_idx] = 1.0
    mk = np.zeros((128, NS, 1, NG), np.float32)
    mk[pe_idx, s_idx, 0, g] = 1.0
    mkb = np.zeros((128, NS, NG, RC, BP), np.float16)
    mkb[pe_idx, s_idx, g] = 1.0
    return wA, wB, mk, mkb


def _build_program(n_iters: int):
    import concourse.bacc as bacc
    import concourse.hw_specs as hw_specs
    # Force every activation onto the one table set containing all our
    # functions (ln/exp/sign/abs); otherwise the chooser ping-pongs between
    # sets and reloads tables (~2.7us per reload).
    if not getattr(bacc, "_act_tables_pinned", False):
        _orig_get = hw_specs.get_activation_tables

        def _pinned(arch):
            tabs = _orig_get(arch)
            keep = "natural_log_exp_and_others"
            if keep in tabs:
                tabs = {k: (v if k == keep else set()) for k, v in tabs.items()}
            return tabs

        bacc.get_activation_tables = _pinned
        bacc._act_tables_pinned = True
    import concourse.mybir as mybir
    import concourse.tile as tile

    f32 = mybir.dt.float32
    f16 = mybir.dt.float16
    i32 = mybir.dt.int32
    AF = mybir.ActivationFunctionType
    ALU = mybir.AluOpType

    nc = bacc.Bacc("TRN2", target_bir_lowering=False, debug=False)

    llr_d = nc.declare_dram_parameter("llr", [BP, N], f32, isOutput=False)
    wA_d = nc.declare_dram_parameter("wA", [128, NS * 128], f16, isOutput=False)
    wB_d = nc.declare_dram_parameter("wB", [128, NS * 128], f16, isOutput=False)
    mkb_d = nc.declare_dram_parameter("mkb", [128, NS * NG * RC * BP], f16,
                                      isOutput=False)
    dec_d = nc.declare_dram_parameter("dec", [BP, N], i32, isOutput=True)

    with tile.TileContext(nc) as tc:
        with tc.tile_pool(name="st", bufs=1) as st, \
             tc.tile_pool(name="ps", bufs=1, space="PSUM") as ps:
            wA = st.tile([128, NS, 128], f16)
            wB = st.tile([128, NS, 128], f16)
            mk = st.tile([128, NS, 1, NG], f32)
            mkb = st.tile([128, NS, NG, RC, BP], f16)  # pre-broadcast fp16 mask
            llr_sb = st.tile([128, NQ, BP], f32)
            x = st.tile([128, NQ, 3, BP], f32)         # per-edge v->c messages
            lt_f = st.tile([128, NS, BP], f32)         # fp32 log-magnitude
            pair = st.tile([128, NS, RC, BP], f16)     # (lt_hi, lt_lo, b) fp16
            R = st.tile([128, NS, NG, RC, BP], f16)    # mask-expanded rhs
            tbf = st.tile([128, RC, BP, NG], f16)      # gather rhs (hi, lo, K)
            Lt_f = st.tile([128, NG, BP], f32)
            Um = st.tile([128, NS, RC, BP, NG], f32)
            G3 = st.tile([128, NS, RC, BP], f32)       # gathered (hi, lo, K) sums
            S = st.tile([128, NQ, 3, BP], f32)
            dlt = st.tile([128, NQ, 3, BP], f32)
            db = st.tile([128, NQ, 3, BP], f32)
            lden = st.tile([128, NQ, 3, BP], f32)
            xp = st.tile([128, NQ, 3, BP], f32)
            ki = st.tile([128, NQ, 3, BP], i32)
            kb = st.tile([128, NQ, 3, BP], i32)
            sgnx = st.tile([128, NQ, 3, BP], f32)
            w = st.tile([128, NQ, 3, BP], f32)
            pd = st.tile([128, NQ, 3, BP], f32)
            L1 = st.tile([128, NQ, 3, BP], f32)
            L2 = st.tile([128, NQ, 3, BP], f32)
            yv = st.tile([128, NQ, 4, BP], f32)        # slots 0..2 = y_j, 3 = llr
            Vf = st.tile([128, NQ, BP], f32)           # llr + sum_j y_j
            sg = st.tile([128, NQ, BP], f32)
            dec_f = st.tile([128, NQ, BP], f32)
            dec_i = st.tile([128, NQ, BP], i32)

            tot_ps = ps.tile([128, NG, RC, BP], f32)   # 120B -> 1 bank
            # gather output in TWO tiles so the mask-select of the first half
            # overlaps the PE still filling the second (deps are per-tile)
            U_a = ps.tile([128, 16, 32], f32)          # s 0..15, one bank
            U_b = ps.tile([128, NS - 16, 32], f32)     # s 16..26, one bank
            ax_ps = ps.tile([128, NQ, 3, BP], f32)
            u_ps = ps.tile([128, NQ, 3, BP], f32)
            ln_ps = ps.tile([128, NQ, 3, BP], f32)

            # ---- init (ordered by when iteration 0 needs each tensor) ----
            nc.sync.dma_start(
                out=llr_sb[:], in_=llr_d.ap().rearrange("b (p q) -> p q b", p=128)
            )
            wA_ap = wA_d.ap().rearrange("p (s c) -> p s c", s=NS)
            wB_ap = wB_d.ap().rearrange("p (s c) -> p s c", s=NS)
            nc.sync.dma_start(out=mkb[:], in_=mkb_d.ap())
            nc.sync.dma_start(out=wA[:, 0:14], in_=wA_ap[:, 0:14])
            nc.sync.dma_start(out=wA[:, 14:NS], in_=wA_ap[:, 14:NS])
            nc.sync.dma_start(out=wB[:, 0:14], in_=wB_ap[:, 0:14])
            nc.sync.dma_start(out=wB[:, 14:NS], in_=wB_ap[:, 14:NS])
            # f32 mask for the PSUM-side select, derived from the fp16 one
            nc.vector.tensor_copy(mk[:, :, 0, :], mkb[:, :, :, 0, 0])
            nc.vector.tensor_copy(
                x[:], llr_sb[:].unsqueeze(2).to_broadcast([128, NQ, 3, BP])
            )
            nc.vector.tensor_copy(yv[:, :, 3, :], llr_sb[:])

            pair_hi = pair[:, :, 0, :]
            pair_lo = pair[:, :, 1, :]
            pair_b = pair[:, :, 2, :]
            ltq = lt_f[:].rearrange("p (q j) b -> p q j b", q=NQ)

            for t in range(n_iters):
                # lt = ln(1e-8 + tanh(|x|/2)) computed exp/ln-only:
                #   u = exp(-|x|); lt = ln(A - B*u) - ln(1 + u)
                # (intermediates go through PSUM: ScalarE's faster port)
                nc.scalar.activation(ax_ps[:], x[:], AF.Abs)
                nc.scalar.activation(u_ps[:], ax_ps[:], AF.Exp, scale=-1.0)
                nc.scalar.activation(ln_ps[:], u_ps[:], AF.Ln, bias=_A32,
                                     scale=-_B32)
                nc.scalar.activation(lden[:], u_ps[:], AF.Ln, bias=1.0)
                nc.vector.tensor_tensor(ltq, ln_ps[:], lden[:], ALU.subtract)
                # fp16 hi/lo split of lt; b = 1 if x < 0 else 0
                nc.vector.tensor_copy(pair_hi, lt_f[:])
                nc.vector.tensor_tensor(pair_lo, lt_f[:], pair_hi, ALU.subtract)
                nc.vector.tensor_scalar(
                    pair_b.rearrange("p (q j) b -> p q j b", q=NQ),
                    x[:], 0.0, None, ALU.is_lt)

                # mask-expand the per-edge rows into their check-group slot
                # (two halves so the scatter matmuls start on half A early)
                for sl, sh in ((0, 14), (14, NS)):
                    nc.vector.tensor_tensor(
                        R[:, sl:sh],
                        pair[:, sl:sh].unsqueeze(2).to_broadcast(
                            [128, sh - sl, NG, RC, BP]),
                        mkb[:, sl:sh],
                        ALU.mult)
                # scatter: tot[pc, g] = sum over edges of check (pc, g)
                for s in range(NS):
                    nc.tensor.matmul(
                        tot_ps[:],
                        wA[:, s, :],
                        R[:, s].rearrange("p g r b -> p (g r b)"),
                        start=(s == 0), stop=(s == NS - 1),
                    )
                # rebuild fp16 hi/lo of the per-check sums for the gather pass
                nc.vector.tensor_reduce(
                    Lt_f[:], tot_ps[:, :, 0:2, :].transpose([0, 1, 3, 2]),
                    axis=mybir.AxisListType.X, op=ALU.add)
                nc.vector.tensor_copy(
                    tbf[:, 0, :, :], Lt_f[:].transpose([0, 2, 1]))
                nc.vector.tensor_tensor(
                    tbf[:, 1, :, :], Lt_f[:].transpose([0, 2, 1]),
                    tbf[:, 0, :, :], ALU.subtract)
                nc.vector.tensor_copy(
                    tbf[:, 2, :, :], tot_ps[:, :, 2, :].transpose([0, 2, 1]))
                # gather: U[pe, s, g] = tot[pc(pe,s), g]  (free order r, b, g)
                tot_flat = tbf[:].rearrange("p r b g -> p (r b g)")
                for s in range(NS):
                    Udst = U_a[:, s, 0:NG * FW] if s < 16 else \
                        U_b[:, s - 16, 0:NG * FW]
                    nc.tensor.matmul(
                        Udst, wB[:, s, :], tot_flat,
                        start=True, stop=True,
                    )
                Um4 = Um[:].rearrange("p s r b g -> p s (r b) g")
                nc.vector.tensor_tensor(
                    Um4[:, 0:16],
                    U_a[:, :, 0:NG * FW].rearrange("p s (rb g) -> p s rb g", g=NG),
                    mk[:, 0:16].to_broadcast([128, 16, RC * BP, NG]), ALU.mult)
                nc.vector.tensor_tensor(
                    Um4[:, 16:NS],
                    U_b[:, :, 0:NG * FW].rearrange("p s (rb g) -> p s rb g", g=NG),
                    mk[:, 16:NS].to_broadcast([128, NS - 16, RC * BP, NG]),
                    ALU.mult)
                nc.vector.tensor_reduce(
                    G3[:], Um4, axis=mybir.AxisListType.X, op=ALU.add)

                # exclusive check sums: sr = Lt - lt, kx = K - b
                G3q = G3[:].rearrange("p (q j) r b -> p q j r b", q=NQ)
                nc.vector.tensor_tensor(S[:], G3q[:, :, :, 0, :], G3q[:, :, :, 1, :],
                                        ALU.add)
                nc.vector.tensor_tensor(dlt[:], S[:], ltq, ALU.subtract)
                nc.vector.tensor_tensor(
                    db[:], G3q[:, :, :, 2, :],
                    pair_b.rearrange("p (q j) b -> p q j b", q=NQ), ALU.subtract)
                nc.scalar.activation(xp[:], dlt[:], AF.Exp)
                # sign of exclusive product: (-1)^kx
                nc.vector.tensor_copy(ki[:], db[:])
                nc.vector.tensor_scalar(kb[:], ki[:], 1, None, ALU.bitwise_and)
                nc.vector.tensor_scalar(sgnx[:], kb[:], -2.0, 1.0, ALU.mult, ALU.add)
                # y = ln(1 + pd) - ln(1 - pd), pd = sgn * (min(xp, 1) - 2e-7)
                # (clamp: fp16 hi/lo transport noise can push xp past 1 +- the
                #  2e-7 guard; the exact product never exceeds 1)
                nc.vector.tensor_scalar(w[:], xp[:], 1.0, -2e-7, ALU.min, ALU.add)
                nc.vector.tensor_tensor(pd[:], w[:], sgnx[:], ALU.mult)
                nc.scalar.activation(L1[:], pd[:], AF.Ln, bias=1.0)
                nc.scalar.activation(L2[:], pd[:], AF.Ln, bias=1.0, scale=-1.0)
                nc.vector.tensor_tensor(yv[:, :, 0:3, :], L1[:], L2[:], ALU.subtract)

                # variable side is local: Vf = llr + sum_j y_j
                nc.vector.tensor_reduce(
                    Vf[:], yv[:].transpose([0, 1, 3, 2]),
                    axis=mybir.AxisListType.X, op=ALU.add)
                if t < n_iters - 1:
                    nc.vector.tensor_tensor(
                        x[:],
                        Vf[:].unsqueeze(2).to_broadcast([128, NQ, 3, BP]),
                        yv[:, :, 0:3, :], ALU.subtract)
                else:
                    nc.scalar.activation(sg[:], Vf[:], AF.Sign)
                    nc.vector.tensor_scalar(dec_f[:], sg[:], -0.5, 0.5,
                                            ALU.mult, ALU.add)
                    nc.vector.tensor_copy(dec_i[:], dec_f[:])
                    nc.sync.dma_start(
                        out=dec_d.ap().rearrange("b (p q) -> p q b", p=128),
                        in_=dec_i[:],
                    )
    nc.compile()
    return nc


_PROGRAM_CACHE = {}


def _get_program(n_iters: int):
    if n_iters not in _PROGRAM_CACHE:
        _PROGRAM_CACHE[n_iters] = _build_program(n_iters)
    return _PROGRAM_CACHE[n_iters]


def _make_in_maps(llr_in, H_sumC_to_V, H_xe_v_sumc_to_y):
    llr = np.ascontiguousarray(np.asarray(llr_in, dtype=np.float32))
    assert llr.shape == (B, N)
    wA, wB, mk, mkb = _derive_structure(H_sumC_to_V, H_xe_v_sumc_to_y)
    wA = np.ascontiguousarray(wA.reshape(128, NS * 128))
    wB = np.ascontiguousarray(wB.reshape(128, NS * 128))
    mkb = np.ascontiguousarray(mkb.reshape(128, NS * NG * RC * BP))
    return [
        {
            "llr": np.ascontiguousarray(llr[c * BP:(c + 1) * BP]),
            "wA": wA,
            "wB": wB,
            "mkb": mkb,
        }
        for c in range(NCORES)
    ]


def kernel(llr_in, H_x_to_xe0, H_sumC_to_V, H_sumV_to_C, H_xe_v_sumc_to_y,
           bp_iter_num, **_unused):
    from concourse.bass_utils import run_bass_kernel_spmd

    n_iters = int(np.asarray(bp_iter_num))
    nc = _get_program(n_iters)
    in_maps = _make_in_maps(llr_in, H_sumC_to_V, H_xe_v_sumc_to_y)
    res = run_bass_kernel_spmd(nc, in_maps, list(range(NCORES)))
    out = np.concatenate([res.results[c]["dec"] for c in range(NCORES)], axis=0)
    return out.astype(np.int32)
